# revision 24
# baseline (speedup 1.0000x reference)
"""Conv4d (B=2, Ci=32, Co=64, 16^4 spatial, k=3^4, stride 1, pad 1) on 8
Trainium2 NeuronCores.

Sharding: 8 cores = batch(2) x T-quarters(4). Each core computes
out[64co, 4t, 16d, 16h, 16w] for its (b, t-quarter).

Per-core layout: 6 per-frame SBUF x tiles [128, 6d*324] bf16 where
partition group r in {0..3} holds ci=32 channels of the padded input
restricted to the D-halo window [4r, 4r+6), planes flattened as
18x18=324. Weights replicated into all 4 partition groups (bf16,
replication done host-side so one DMA trigger suffices).

PE array runs as 8 concurrent 32x64 tiles: tile_position=(32r, 64c)
with r = output-D-quarter, c = d-pair within the quarter. Output-T
frames are processed in pairs: each (r, c) subarray accumulates TWO
output tiles (to=2tp, 2tp+1) in two PSUM half-banks (bank 4k+r holds
the (r, c=0, k) accumulator in partitions 0-63 and (r, c=1, k) in
64-127), sharing one LDWEIGHTS per tap: pair order is pinned with
no-sync deps and the redundant second load is stripped by
_dedup_ldweights after scheduling. 81 tap matmuls (K=32ci, M=64co,
N=512=2d*16h*16w) per accumulator; the epilogue adds bias to a whole
bank [128, 512] in one DVE/ACT op and moves it with one DMA into a
bank-shaped DRAM layout. Input DMAs ride one ring in priority order
(w-half0, x0, x1 first; the rest gated behind the first matmul via a
sync dep so they don't steal HBM bandwidth from the critical tiles).
"""
import sys

sys.path.insert(0, "/opt/trn_rl_repo")
import numpy as np
import ml_dtypes

N_CORES = 8
TAPS = [(kt, kd, kh, kw) for kt in range(3) for kd in range(3)
        for kh in range(3) for kw in range(3)]

_NC = None


def _dedup_ldweights(nc, mybir):
    """Remove back-to-back duplicate LDWEIGHTS per PE tile.

    The paired matmuls (same tap, two output-T frames) emit identical
    weight loads for the same tile_position; the second is redundant —
    the subarray's cells still hold the tap weights (our no-sync deps
    pin the pair's order, and no other LDWEIGHTS targets this tile in
    between). Only waits/updates-free loads are dropped.
    """
    removed = 0
    for f in nc.m.functions:
        for blk in f.blocks:
            last = {}
            keep = []
            for ins in blk.instructions:
                if isinstance(ins, mybir.InstLdweights):
                    key = ins.tile_position
                    sig = (ins.ins[0].concise(), ins.tile_size,
                           ins.perf_mode, ins.is_transpose)
                    si = ins.sync_info
                    clean = (si is None) or (len(si.on_wait) == 0
                                             and len(si.on_update) == 0)
                    if last.get(key) == sig and clean:
                        removed += 1
                        continue
                    last[key] = sig
                keep.append(ins)
            blk.instructions = keep
    return removed


def _build():
    global _NC
    if _NC is not None:
        return _NC
    import concourse.bacc as bacc
    import concourse.tile as tile
    from concourse import mybir

    f32 = mybir.dt.float32
    bf16 = mybir.dt.bfloat16
    NOSYNC = mybir.DependencyInfo.NO_SYNC_ONLY

    nc = bacc.Bacc("TRN2", debug=False, target_bir_lowering=False,
                   num_devices=N_CORES)
    xq = nc.dram_tensor("xq", [128, 6 * 1944], bf16, kind="ExternalInput")
    wq = nc.dram_tensor("wq", [128, 81 * 64], bf16, kind="ExternalInput")
    bq = nc.dram_tensor("biasq", [128, 1], f32, kind="ExternalInput")
    # out[p, (to*4+r)*512 + dd*256 + e]: p<64 -> (co=p, d=4r+dd),
    # p>=64 -> (co=p-64, d=4r+2+dd) — lets one DMA move a whole PSUM bank
    out = nc.dram_tensor("out", [128, 8192], f32, kind="ExternalOutput")

    with tile.TileContext(nc) as tc:
        with tc.tile_pool(name="xp", bufs=1) as xp, \
             tc.tile_pool(name="wp", bufs=1) as wp, \
             tc.tile_pool(name="op", bufs=16) as op_, \
             tc.tile_pool(name="pp", bufs=8, space="PSUM") as pp:
            # weight halves as separate tiles: the first 40 taps' loads
            # only wait on the first 0.66MB, not the full weight DMA
            WSPLIT = 40 * 64
            wtiles = [wp.tile([128, WSPLIT], bf16, name="wt0"),
                      wp.tile([128, 5184 - WSPLIT], bf16, name="wt1")]
            xtiles = [xp.tile([128, 1944], bf16, name=f"xt{tf}")
                      for tf in range(6)]
            # all input DMAs on one ring in priority order: per-queue
            # FIFO gives the first-needed tiles full bandwidth
            nc.gpsimd.dma_start(wtiles[0][:], wq.ap()[:, 0:WSPLIT])
            nc.gpsimd.dma_start(xtiles[0][:], xq.ap()[:, 0:1944])
            nc.gpsimd.dma_start(xtiles[1][:], xq.ap()[:, 1944:2 * 1944])
            # later-needed inputs held back behind the first matmul so
            # they don't steal HBM bandwidth from the critical tiles
            delayed_dmas = [nc.gpsimd.dma_start(wtiles[1][:],
                                                wq.ap()[:, WSPLIT:5184])]
            for tf in range(2, 6):
                delayed_dmas.append(nc.gpsimd.dma_start(
                    xtiles[tf][:], xq.ap()[:, tf * 1944:(tf + 1) * 1944]))
            btile = wp.tile([128, 1], f32)
            nc.scalar.dma_start(btile[:], bq.ap()[:])

            def lhsT_of(i, r):
                wh = int(i >= 40)
                col = i * 64 - wh * WSPLIT
                return wtiles[wh][32 * r:32 * r + 32, col:col + 64]

            xts = [xt.rearrange("p (d h w) -> p d h w", d=6, h=18, w=18)
                   for xt in xtiles]

            last = {}
            for tp in range(2):
                # bank 4k+r: lower half <- (r, c=0, to=2tp+k),
                #            upper half <- (r, c=1, to=2tp+k)
                ps = [pp.tile([128, 512], f32, tag="ps",
                              name=f"ps_{tp}_{j}") for j in range(8)]
                for i, (kt, kd, kh, kw) in enumerate(TAPS):
                    for c in range(2):
                        for r in range(4):
                            lhsT = lhsT_of(i, r)
                            for k in range(2):
                                xv = xts[2 * tp + k + kt]
                                rhs = xv[32 * r:32 * r + 32,
                                         2 * c + kd: 2 * c + kd + 2,
                                         kh:kh + 16, kw:kw + 16]
                                m = nc.tensor.matmul(
                                    ps[4 * k + r][64 * c:64 * c + 64, :],
                                    lhsT, rhs, start=(i == 0), stop=(i == 80),
                                    tile_position=(32 * r, 64 * c))
                                prev = last.get((r, c))
                                if prev is not None:
                                    m.ins.add_dependency(prev.ins.name, NOSYNC)
                                if not last:
                                    sync = mybir.DependencyInfo.SYNC_ONLY
                                    for d in delayed_dmas:
                                        d.ins.add_dependency(m.ins.name, sync)
                                last[(r, c)] = m
                # full-bank epilogue: one DVE/ACT op and one DMA per
                # bank moves both column halves (same engine cost as
                # half a bank)
                for k in range(2):
                    for r in range(4):
                        to = 2 * tp + k
                        o = op_.tile([128, 512], f32, tag="ob",
                                     name=f"o_{tp}_{r}_{k}")
                        psl = ps[4 * k + r][:, :]
                        if (r + k) % 2 == 0:
                            nc.vector.tensor_scalar_add(
                                o[:], psl, btile[:, 0:1])
                        else:
                            nc.scalar.activation(
                                o[:], psl,
                                mybir.ActivationFunctionType.Identity,
                                bias=btile[:, 0:1])
                        blk = to * 4 + r
                        # tail triggers on the idle sync ring: scalar is
                        # busy with its 4 evacs there
                        deng = nc.gpsimd if tp == 0 else nc.sync
                        deng.dma_start(
                            out.ap()[:, blk * 512:(blk + 1) * 512], o[:])
    _dedup_ldweights(nc, mybir)
    nc.compile()
    _NC = nc
    return nc


def _prep_inputs(x, weight, bias):
    x = np.asarray(x, dtype=np.float32)
    weight = np.asarray(weight, dtype=np.float32)
    bias = np.asarray(bias, dtype=np.float32)

    w9 = weight.reshape(64, 32, 81).transpose(2, 1, 0)  # [tap, ci, co]
    warr = np.ascontiguousarray(w9.transpose(1, 0, 2)).reshape(32, 81 * 64)
    wq = np.tile(warr, (4, 1)).astype(ml_dtypes.bfloat16)  # [128, 5184]
    bq = np.tile(bias.reshape(64, 1), (2, 1)).astype(np.float32)  # [128, 1]

    in_maps = []
    for b in range(2):
        xpad = np.pad(x[b], ((0, 0), (1, 1), (1, 1), (1, 1), (1, 1)))
        for tq in range(4):
            xt = xpad[:, 4 * tq:4 * tq + 6]  # [32ci, 6t, 18d, 18h, 18w]
            # frame-major layout: xqc[p, tf*1944 + (d*324 + h*18 + w)]
            # with partition group r holding D window [4r, 4r+6)
            xqc = np.empty((128, 6 * 1944), np.float32)
            for r in range(4):
                xqc[32 * r:32 * r + 32] = \
                    xt[:, :, 4 * r:4 * r + 6].reshape(32, 6, 1944) \
                    .reshape(32, -1)
            in_maps.append({"xq": xqc.astype(ml_dtypes.bfloat16),
                            "wq": wq, "biasq": bq})
    return in_maps


def run_spmd(x, weight, bias, trace=False, trace_cores=None, tmpdir=None):
    """Returns (output ndarray, BassKernelResults)."""
    from concourse.bass_utils import run_bass_kernel_spmd
    nc = _build()
    in_maps = _prep_inputs(x, weight, bias)
    res = run_bass_kernel_spmd(nc, in_maps, core_ids=list(range(N_CORES)),
                               trace=trace, trace_cores=trace_cores,
                               tmpdir=tmpdir)
    out = np.empty((2, 64, 16, 16, 16, 16), np.float32)
    for c in range(N_CORES):
        b, tq = c // 4, c % 4
        # [h, co, to, r, dd, 16, 16] -> d = 4r + 2h + dd
        arr = res.results[c]["out"].reshape(2, 64, 4, 4, 2, 16, 16)
        out[b, :, 4 * tq:4 * tq + 4] = \
            arr.transpose(1, 2, 3, 0, 4, 5, 6).reshape(64, 4, 16, 16, 16)
    return out, res


def kernel(x, weight, bias):
    out, _ = run_spmd(x, weight, bias)
    return out


# revision 25
# speedup vs baseline: 1.0072x; 1.0072x over previous
"""Conv4d (B=2, Ci=32, Co=64, 16^4 spatial, k=3^4, stride 1, pad 1) on 8
Trainium2 NeuronCores.

Sharding: 8 cores = batch(2) x T-quarters(4). Each core computes
out[64co, 4t, 16d, 16h, 16w] for its (b, t-quarter).

Per-core layout: 6 per-frame SBUF x tiles [128, 6d*324] bf16 where
partition group r in {0..3} holds ci=32 channels of the padded input
restricted to the D-halo window [4r, 4r+6), planes flattened as
18x18=324. Weights replicated into all 4 partition groups (bf16,
replication done host-side so one DMA trigger suffices).

PE array runs as 8 concurrent 32x64 tiles: tile_position=(32r, 64c)
with r = output-D-quarter, c = d-pair within the quarter. Output-T
frames are processed in pairs: each (r, c) subarray accumulates TWO
output tiles (to=2tp, 2tp+1) in two PSUM half-banks (bank 4k+r holds
the (r, c=0, k) accumulator in partitions 0-63 and (r, c=1, k) in
64-127), sharing one LDWEIGHTS per tap: pair order is pinned with
no-sync deps and the redundant second load is stripped by
_dedup_ldweights after scheduling. 81 tap matmuls (K=32ci, M=64co,
N=512=2d*16h*16w) per accumulator; the epilogue adds bias to a whole
bank [128, 512] in one DVE/ACT op and moves it with one DMA into a
bank-shaped DRAM layout. Input DMAs ride one ring in priority order
(w-half0, x0, x1 first; the rest gated behind the first matmul via a
sync dep so they don't steal HBM bandwidth from the critical tiles).
"""
import sys

sys.path.insert(0, "/opt/trn_rl_repo")
import numpy as np
import ml_dtypes

N_CORES = 8
TAPS = [(kt, kd, kh, kw) for kt in range(3) for kd in range(3)
        for kh in range(3) for kw in range(3)]

_NC = None


def _dedup_ldweights(nc, mybir):
    """Remove back-to-back duplicate LDWEIGHTS per PE tile.

    The paired matmuls (same tap, two output-T frames) emit identical
    weight loads for the same tile_position; the second is redundant —
    the subarray's cells still hold the tap weights (our no-sync deps
    pin the pair's order, and no other LDWEIGHTS targets this tile in
    between). Only waits/updates-free loads are dropped.
    """
    removed = 0
    for f in nc.m.functions:
        for blk in f.blocks:
            last = {}
            keep = []
            for ins in blk.instructions:
                if isinstance(ins, mybir.InstLdweights):
                    key = ins.tile_position
                    sig = (ins.ins[0].concise(), ins.tile_size,
                           ins.perf_mode, ins.is_transpose)
                    si = ins.sync_info
                    clean = (si is None) or (len(si.on_wait) == 0
                                             and len(si.on_update) == 0)
                    if last.get(key) == sig and clean:
                        removed += 1
                        continue
                    last[key] = sig
                keep.append(ins)
            blk.instructions = keep
    return removed


def _build():
    global _NC
    if _NC is not None:
        return _NC
    import concourse.bacc as bacc
    import concourse.tile as tile
    from concourse import mybir

    f32 = mybir.dt.float32
    bf16 = mybir.dt.bfloat16
    NOSYNC = mybir.DependencyInfo.NO_SYNC_ONLY

    nc = bacc.Bacc("TRN2", debug=False, target_bir_lowering=False,
                   num_devices=N_CORES)
    xq = nc.dram_tensor("xq", [128, 6 * 1944], bf16, kind="ExternalInput")
    wq = nc.dram_tensor("wq", [128, 81 * 64], bf16, kind="ExternalInput")
    bq = nc.dram_tensor("biasq", [128, 1], f32, kind="ExternalInput")
    # out[p, (to*4+r)*512 + dd*256 + e]: p<64 -> (co=p, d=4r+dd),
    # p>=64 -> (co=p-64, d=4r+2+dd) — lets one DMA move a whole PSUM bank
    out = nc.dram_tensor("out", [128, 8192], f32, kind="ExternalOutput")

    with tile.TileContext(nc) as tc:
        with tc.tile_pool(name="xp", bufs=1) as xp, \
             tc.tile_pool(name="wp", bufs=1) as wp, \
             tc.tile_pool(name="op", bufs=16) as op_, \
             tc.tile_pool(name="pp", bufs=8, space="PSUM") as pp:
            # weight halves as separate tiles: the first 40 taps' loads
            # only wait on the first 0.66MB, not the full weight DMA
            WSPLIT = 40 * 64
            wtiles = [wp.tile([128, WSPLIT], bf16, name="wt0"),
                      wp.tile([128, 5184 - WSPLIT], bf16, name="wt1")]
            xtiles = [xp.tile([128, 1944], bf16, name=f"xt{tf}")
                      for tf in range(6)]
            # all input DMAs on one ring in priority order: per-queue
            # FIFO gives the first-needed tiles full bandwidth
            nc.gpsimd.dma_start(wtiles[0][:], wq.ap()[:, 0:WSPLIT])
            nc.gpsimd.dma_start(xtiles[0][:], xq.ap()[:, 0:1944])
            nc.gpsimd.dma_start(xtiles[1][:], xq.ap()[:, 1944:2 * 1944])
            # later-needed inputs held back behind the first matmul so
            # they don't steal HBM bandwidth from the critical tiles
            delayed_dmas = [nc.gpsimd.dma_start(wtiles[1][:],
                                                wq.ap()[:, WSPLIT:5184])]
            for tf in range(2, 6):
                delayed_dmas.append(nc.gpsimd.dma_start(
                    xtiles[tf][:], xq.ap()[:, tf * 1944:(tf + 1) * 1944]))
            btile = wp.tile([128, 1], f32)
            nc.scalar.dma_start(btile[:], bq.ap()[:])

            def lhsT_of(i, r):
                wh = int(i >= 40)
                col = i * 64 - wh * WSPLIT
                return wtiles[wh][32 * r:32 * r + 32, col:col + 64]

            xts = [xt.rearrange("p (d h w) -> p d h w", d=6, h=18, w=18)
                   for xt in xtiles]

            last = {}
            for tp in range(2):
                # bank 4k+r: lower half <- (r, c=0, to=2tp+k),
                #            upper half <- (r, c=1, to=2tp+k)
                ps = [pp.tile([128, 512], f32, tag="ps",
                              name=f"ps_{tp}_{j}") for j in range(8)]
                for i, (kt, kd, kh, kw) in enumerate(TAPS):
                    for c in range(2):
                        for r in range(4):
                            lhsT = lhsT_of(i, r)
                            for k in range(2):
                                xv = xts[2 * tp + k + kt]
                                rhs = xv[32 * r:32 * r + 32,
                                         2 * c + kd: 2 * c + kd + 2,
                                         kh:kh + 16, kw:kw + 16]
                                m = nc.tensor.matmul(
                                    ps[4 * k + r][64 * c:64 * c + 64, :],
                                    lhsT, rhs, start=(i == 0), stop=(i == 80),
                                    tile_position=(32 * r, 64 * c))
                                prev = last.get((r, c))
                                if prev is not None:
                                    m.ins.add_dependency(prev.ins.name, NOSYNC)
                                if not last:
                                    sync = mybir.DependencyInfo.SYNC_ONLY
                                    for d in delayed_dmas:
                                        d.ins.add_dependency(m.ins.name, sync)
                                last[(r, c)] = m
                # full-bank epilogue: one DVE/ACT op and one DMA per
                # bank moves both column halves (same engine cost as
                # half a bank)
                for k in range(2):
                    for r in range(4):
                        to = 2 * tp + k
                        o = op_.tile([128, 512], f32, tag="ob",
                                     name=f"o_{tp}_{r}_{k}")
                        psl = ps[4 * k + r][:, :]
                        if (r + k) % 2 == 0:
                            nc.vector.tensor_scalar_add(
                                o[:], psl, btile[:, 0:1])
                        else:
                            nc.scalar.activation(
                                o[:], psl,
                                mybir.ActivationFunctionType.Identity,
                                bias=btile[:, 0:1])
                        blk = to * 4 + r
                        if tp == 0:
                            deng = nc.gpsimd
                        else:
                            deng = nc.sync if k == 0 else nc.scalar
                        deng.dma_start(
                            out.ap()[:, blk * 512:(blk + 1) * 512], o[:])
    _dedup_ldweights(nc, mybir)
    nc.compile()
    _NC = nc
    return nc


def _prep_inputs(x, weight, bias):
    x = np.asarray(x, dtype=np.float32)
    weight = np.asarray(weight, dtype=np.float32)
    bias = np.asarray(bias, dtype=np.float32)

    w9 = weight.reshape(64, 32, 81).transpose(2, 1, 0)  # [tap, ci, co]
    warr = np.ascontiguousarray(w9.transpose(1, 0, 2)).reshape(32, 81 * 64)
    wq = np.tile(warr, (4, 1)).astype(ml_dtypes.bfloat16)  # [128, 5184]
    bq = np.tile(bias.reshape(64, 1), (2, 1)).astype(np.float32)  # [128, 1]

    in_maps = []
    for b in range(2):
        xpad = np.pad(x[b], ((0, 0), (1, 1), (1, 1), (1, 1), (1, 1)))
        for tq in range(4):
            xt = xpad[:, 4 * tq:4 * tq + 6]  # [32ci, 6t, 18d, 18h, 18w]
            # frame-major layout: xqc[p, tf*1944 + (d*324 + h*18 + w)]
            # with partition group r holding D window [4r, 4r+6)
            xqc = np.empty((128, 6 * 1944), np.float32)
            for r in range(4):
                xqc[32 * r:32 * r + 32] = \
                    xt[:, :, 4 * r:4 * r + 6].reshape(32, 6, 1944) \
                    .reshape(32, -1)
            in_maps.append({"xq": xqc.astype(ml_dtypes.bfloat16),
                            "wq": wq, "biasq": bq})
    return in_maps


def run_spmd(x, weight, bias, trace=False, trace_cores=None, tmpdir=None):
    """Returns (output ndarray, BassKernelResults)."""
    from concourse.bass_utils import run_bass_kernel_spmd
    nc = _build()
    in_maps = _prep_inputs(x, weight, bias)
    res = run_bass_kernel_spmd(nc, in_maps, core_ids=list(range(N_CORES)),
                               trace=trace, trace_cores=trace_cores,
                               tmpdir=tmpdir)
    out = np.empty((2, 64, 16, 16, 16, 16), np.float32)
    for c in range(N_CORES):
        b, tq = c // 4, c % 4
        # [h, co, to, r, dd, 16, 16] -> d = 4r + 2h + dd
        arr = res.results[c]["out"].reshape(2, 64, 4, 4, 2, 16, 16)
        out[b, :, 4 * tq:4 * tq + 4] = \
            arr.transpose(1, 2, 3, 0, 4, 5, 6).reshape(64, 4, 16, 16, 16)
    return out, res


def kernel(x, weight, bias):
    out, _ = run_spmd(x, weight, bias)
    return out
